# revision 24
# baseline (speedup 1.0000x reference)
"""Trainium2 Bass kernel for nn_MoE_66803921322559.

Top-2-of-16 MoE (T=2048 tokens, D=1024, INTER=512) + shared expert
(SHARED_INTER=1024), expert-parallel over 8 NeuronCores:

  - core c owns experts (2c, 2c+1); gate computed on-device (double-bf16
    logits = xh@gh + xl@gh + xh@gl, exact enough that top-2 selection
    matches fp32; sigmoid/normalize for combine weights)
  - routed experts + shared-expert slice computed in bf16 on device;
    shared expert is scheduled first so the gate pipeline latency hides
  - partial outputs y [D, T] summed across cores with 4 d-chunked
    ReduceScatters overlapped with the y-phase matmuls; host reassembles.
"""

import os
import sys
import types

import numpy as np

sys.path.insert(0, "/opt/trn_rl_repo")

import ml_dtypes

BF = ml_dtypes.bfloat16

B, S, DIM = 2, 1024, 1024
E, K, INTER = 16, 2, 512
T = B * S
N_CORES = 8
EPC = E // N_CORES          # experts per core
SIC = 2 * INTER // N_CORES  # shared-inter slice per core (128)

KD = DIM // 128             # 8 contraction chunks over D
NT = T // 512               # 4 token chunks of 512
MI = INTER // 128           # 4 inter chunks per expert
NTILE = T // 128            # 16 token tiles of 128
NCH = 4                     # ReduceScatter d-chunks


def _install_ntff_hook():
    """Provide antenv.axon_hooks (missing in this container) so
    run_bass_kernel_spmd(trace=True) can capture NTFF profiles via axon."""
    try:
        import antenv
        if hasattr(antenv, "axon_hooks"):
            return
        from trn_agent_boot.trn_boot import _ntff_profile_via_ctypes
        hook = _ntff_profile_via_ctypes("/opt/axon/libaxon_pjrt.so")
        mod = types.ModuleType("antenv.axon_hooks")
        mod._hook = hook
        mod.get_axon_ntff_profile_hook = lambda: mod._hook
        mod.set_axon_ntff_profile_hook = lambda h: setattr(mod, "_hook", h)
        sys.modules["antenv.axon_hooks"] = mod
        antenv.axon_hooks = mod
    except Exception:
        pass


_install_ntff_hook()

from concourse import bacc, bass, mybir, tile  # noqa: E402
from concourse.bass_utils import run_bass_kernel_spmd  # noqa: E402
from concourse.masks import make_identity  # noqa: E402

F32 = mybir.dt.float32
BF16 = mybir.dt.bfloat16
AF = mybir.ActivationFunctionType
ALU = mybir.AluOpType

last_exec_time_ns = None
_cached = {}


def _build():
    nc = bacc.Bacc("TRN2", target_bir_lowering=False, debug=False,
                   num_devices=N_CORES)

    xtb_d = nc.dram_tensor("xtb", [DIM, T], BF16, kind="ExternalInput").ap()
    xtl_d = nc.dram_tensor("xtl", [DIM, T], BF16, kind="ExternalInput").ap()
    ghgl_d = nc.dram_tensor("ghgl", [DIM, 2 * E], BF16, kind="ExternalInput").ap()
    w1t_d = nc.dram_tensor("w1t", [EPC, DIM, INTER], BF16, kind="ExternalInput").ap()
    w3t_d = nc.dram_tensor("w3t", [EPC, DIM, INTER], BF16, kind="ExternalInput").ap()
    w2t_d = nc.dram_tensor("w2t", [EPC, INTER, DIM], BF16, kind="ExternalInput").ap()
    sw1t_d = nc.dram_tensor("sw1t", [DIM, SIC], BF16, kind="ExternalInput").ap()
    sw3t_d = nc.dram_tensor("sw3t", [DIM, SIC], BF16, kind="ExternalInput").ap()
    sw2t_d = nc.dram_tensor("sw2t", [SIC, DIM], BF16, kind="ExternalInput").ap()
    out_d = nc.dram_tensor("out", [DIM // N_CORES, T], BF16,
                           kind="ExternalOutput").ap()

    with tile.TileContext(nc) as tc:
        with (
            tc.tile_pool(name="wpool", bufs=1) as wp,
            tc.tile_pool(name="work", bufs=3) as wk,
            tc.tile_pool(name="psum", bufs=2, space="PSUM") as pp,
            tc.tile_pool(name="psc", bufs=2, space="PSUM") as pscp,
            tc.tile_pool(name="dram", bufs=1, space="DRAM") as dp,
        ):
            # ---- persistent SBUF loads, in consumption order ---------------
            # sync ring: gate inputs first; scalar ring: expert weights
            xtb, xtl, ghs = [], [], []
            for k in range(KD):
                ksl = slice(k * 128, (k + 1) * 128)
                t_ = wp.tile([128, T], BF16, tag=f"xtb{k}", name=f"xtb{k}")
                nc.sync.dma_start(out=t_[:], in_=xtb_d[ksl, :])
                xtb.append(t_)
                t_ = wp.tile([128, 2 * E], BF16, tag=f"gh{k}", name=f"gh{k}")
                nc.sync.dma_start(out=t_[:], in_=ghgl_d[ksl, :])
                ghs.append(t_)
            for k in range(KD):
                ksl = slice(k * 128, (k + 1) * 128)
                t_ = wp.tile([128, T], BF16, tag=f"xtl{k}", name=f"xtl{k}")
                nc.sync.dma_start(out=t_[:], in_=xtl_d[ksl, :])
                xtl.append(t_)
            sw1s, sw3s = [], []
            for k in range(KD):
                ksl = slice(k * 128, (k + 1) * 128)
                t_ = wp.tile([128, SIC], BF16, tag=f"sw1s{k}", name=f"sw1s{k}")
                nc.scalar.dma_start(out=t_[:], in_=sw1t_d[ksl, :])
                sw1s.append(t_)
                t_ = wp.tile([128, SIC], BF16, tag=f"sw3s{k}", name=f"sw3s{k}")
                nc.scalar.dma_start(out=t_[:], in_=sw3t_d[ksl, :])
                sw3s.append(t_)
            w1s = [[None] * KD for _ in range(EPC)]
            w3s = [[None] * KD for _ in range(EPC)]
            for e in range(EPC):
                for k in range(KD):
                    ksl = slice(k * 128, (k + 1) * 128)
                    t_ = wp.tile([128, INTER], BF16, tag=f"w1s{e}_{k}",
                                 name=f"w1s{e}_{k}")
                    nc.scalar.dma_start(out=t_[:], in_=w1t_d[e, ksl, :])
                    w1s[e][k] = t_
                    t_ = wp.tile([128, INTER], BF16, tag=f"w3s{e}_{k}",
                                 name=f"w3s{e}_{k}")
                    nc.scalar.dma_start(out=t_[:], in_=w3t_d[e, ksl, :])
                    w3s[e][k] = t_
            ident = wp.tile([128, 128], F32, tag="ident")
            make_identity(nc, ident[:])

            # ---- gate: double-bf16 logits, [gh|gl] packed stationary -------
            # rows 0:16 = gh-terms, 16:32 = gl-terms; summed after transpose
            logits = wp.tile([2 * E, T], F32, tag="logits")
            for n in range(NT):
                nsl = slice(n * 512, (n + 1) * 512)
                psc = pscp.tile([2 * E, 512], F32, tag="pg", name="psc")
                for k in range(KD):
                    nc.tensor.matmul(psc[:], lhsT=ghs[k][:], rhs=xtb[k][:, nsl],
                                     start=(k == 0), stop=False)
                for k in range(KD):
                    nc.tensor.matmul(psc[:], lhsT=ghs[k][:], rhs=xtl[k][:, nsl],
                                     start=False, stop=(k == KD - 1))
                nc.vector.tensor_copy(out=logits[:, nsl], in_=psc[:])

            # ---- shared expert slice (no gate dependency): emitted before
            # the gate's transpose/DVE chain so PE stays busy ----------------
            hss = wp.tile([128, T], BF16, tag="hss")
            for n in range(NT):
                nsl = slice(n * 512, (n + 1) * 512)
                ps1 = pp.tile([128, 512], F32, tag="ps1", name="ps1")
                ps3 = pp.tile([128, 512], F32, tag="ps3", name="ps3")
                for k in range(KD):
                    nc.tensor.matmul(ps1[:], lhsT=sw1s[k][:], rhs=xtb[k][:, nsl],
                                     start=(k == 0), stop=(k == KD - 1))
                for k in range(KD):
                    nc.tensor.matmul(ps3[:], lhsT=sw3s[k][:], rhs=xtb[k][:, nsl],
                                     start=(k == 0), stop=(k == KD - 1))
                t1 = wk.tile([128, 512], BF16, tag="t1")
                nc.scalar.activation(out=t1[:], in_=ps1[:], func=AF.Silu)
                nc.vector.tensor_tensor(out=hss[:, nsl], in0=t1[:], in1=ps3[:],
                                        op=ALU.mult)

            # batched gate: transpose logits, top-2, combine weights
            lg_all = wp.tile([128, NTILE * E], F32, tag="lg_all")
            mx_all = wp.tile([128, NTILE * 8], F32, tag="mx_all")
            for i in range(NTILE):
                isl = slice(i * 128, (i + 1) * 128)
                ptr = pscp.tile([128, 2 * E], F32, tag="pg", name="ptr")
                nc.tensor.transpose(out=ptr[:], in_=logits[:, isl],
                                    identity=ident[:2 * E, :2 * E])
                lg32 = wk.tile([128, 2 * E], F32, tag="lg32")
                nc.vector.tensor_copy(out=lg32[:], in_=ptr[:])
                nc.vector.tensor_tensor(out=lg_all[:, i * E:(i + 1) * E],
                                        in0=lg32[:, 0:E], in1=lg32[:, E:2 * E],
                                        op=ALU.add)
            for i in range(NTILE):
                nc.vector.max(out=mx_all[:, i * 8:(i + 1) * 8],
                              in_=lg_all[:, i * E:(i + 1) * E])
            # denominator = sigmoid(l1) + sigmoid(l2); rec = 1/den
            mx3 = mx_all[:].rearrange("p (i c) -> p i c", c=8)
            lg3 = lg_all[:].rearrange("p (i c) -> p i c", c=E)
            s12 = wk.tile([128, 2 * NTILE], F32, tag="s12")
            nc.scalar.activation(out=s12[:], in_=mx3[:, :, 0:2], func=AF.Sigmoid)
            s12v = s12[:].rearrange("p (i c) -> p i c", c=2)
            den = wk.tile([128, NTILE], F32, tag="den")
            nc.vector.tensor_tensor(out=den[:], in0=s12v[:, :, 0],
                                    in1=s12v[:, :, 1], op=ALU.add)
            rec = wk.tile([128, NTILE], F32, tag="rec")
            nc.vector.reciprocal(out=rec[:], in_=den[:])

            wexp = []
            for e in range(EPC):
                sel = wk.tile([128, NTILE], F32, tag=f"sel{e}", name=f"sel{e}")
                nc.vector.tensor_tensor(out=sel[:], in0=lg3[:, :, e],
                                        in1=mx3[:, :, 1], op=ALU.is_ge)
                sg = wk.tile([128, NTILE], F32, tag=f"sg{e}", name=f"sg{e}")
                nc.scalar.activation(out=sg[:], in_=lg3[:, :, e], func=AF.Sigmoid)
                we = wk.tile([128, NTILE], F32, tag=f"wexp{e}", name=f"wexp{e}")
                nc.vector.tensor_tensor(out=we[:], in0=sg[:], in1=sel[:],
                                        op=ALU.mult)
                nc.vector.tensor_tensor(out=we[:], in0=we[:], in1=rec[:],
                                        op=ALU.mult)
                wexp.append(we)

            # transpose per-token weights into rows, then partition-broadcast
            wbc = []
            for e in range(EPC):
                wrow = wp.tile([1, T], BF16, tag=f"wrow{e}", name=f"wrow{e}")
                for i in range(NTILE):
                    pwt = pscp.tile([1, 128], F32, tag="pg", name="pwt")
                    nc.tensor.transpose(out=pwt[:], in_=wexp[e][:, i:i + 1],
                                        identity=ident[:])
                    nc.vector.tensor_copy(out=wrow[:, i * 128:(i + 1) * 128],
                                          in_=pwt[:])
                t_ = wp.tile([128, T], BF16, tag=f"wbc{e}", name=f"wbc{e}")
                nc.gpsimd.partition_broadcast(t_[:], wrow[:, :])
                wbc.append(t_)

            # w2 weights are first needed by the y-phase (~110us in); load
            # them after w1/w3 so the h-phase weight stream gets the early HBM
            # bandwidth
            w2s = [[None] * MI for _ in range(EPC)]
            for e in range(EPC):
                for m in range(MI):
                    msl = slice(m * 128, (m + 1) * 128)
                    t_ = wp.tile([128, DIM], BF16, tag=f"w2s{e}_{m}",
                                 name=f"w2s{e}_{m}")
                    nc.scalar.dma_start(out=t_[:], in_=w2t_d[e, msl, :])
                    w2s[e][m] = t_
            sw2s = wp.tile([128, DIM], BF16, tag="sw2s")
            nc.scalar.dma_start(out=sw2s[:], in_=sw2t_d[:, :])

            # ---- fused pipeline, token-chunk outer: h(n) -> y(n) -> RS(n) --
            hsb = [[wp.tile([128, T], BF16, tag=f"hsb{e}_{m}", name=f"hsb{e}_{m}")
                    for m in range(MI)] for e in range(EPC)]
            y_ch = [dp.tile([DIM, 512], BF16, name=f"y_ch{n}") for n in range(NT)]
            y_rs = [dp.tile([DIM // N_CORES, 512], BF16, name=f"y_rs{n}")
                    for n in range(NT)]
            for n in range(NT):
                nsl = slice(n * 512, (n + 1) * 512)
                # routed experts
                for e in range(EPC):
                    for m in range(MI):
                        msl = slice(m * 128, (m + 1) * 128)
                        ps1 = pp.tile([128, 512], F32, tag="ps1", name="ps1")
                        ps3 = pp.tile([128, 512], F32, tag="ps3", name="ps3")
                        for k in range(KD):
                            nc.tensor.matmul(ps1[:], lhsT=w1s[e][k][:, msl],
                                             rhs=xtb[k][:, nsl],
                                             start=(k == 0), stop=(k == KD - 1))
                        for k in range(KD):
                            nc.tensor.matmul(ps3[:], lhsT=w3s[e][k][:, msl],
                                             rhs=xtb[k][:, nsl],
                                             start=(k == 0), stop=(k == KD - 1))
                        t1 = wk.tile([128, 512], BF16, tag="t1")
                        nc.scalar.activation(out=t1[:], in_=ps1[:], func=AF.Silu)
                        nc.vector.tensor_tensor(out=hsb[e][m][:, nsl], in0=t1[:],
                                                in1=ps3[:], op=ALU.mult)
                # apply gate weights in place, decoupled from the PSUM pipeline
                for e in range(EPC):
                    for m in range(MI):
                        nc.vector.tensor_tensor(out=hsb[e][m][:, nsl],
                                                in0=hsb[e][m][:, nsl],
                                                in1=wbc[e][:, nsl], op=ALU.mult)
                # y for this token chunk, then its ReduceScatter
                for d in range(KD):
                    dsl = slice(d * 128, (d + 1) * 128)
                    psy = pp.tile([128, 512], F32, tag="psy", name="psy")
                    nc.tensor.matmul(psy[:], lhsT=sw2s[:, dsl],
                                     rhs=hss[:, nsl], start=True, stop=False)
                    for e in range(EPC):
                        for m in range(MI):
                            nc.tensor.matmul(
                                psy[:], lhsT=w2s[e][m][:, dsl],
                                rhs=hsb[e][m][:, nsl],
                                start=False,
                                stop=(e == EPC - 1 and m == MI - 1))
                    ysb = wk.tile([128, 512], BF16, tag="ysb")
                    nc.vector.tensor_copy(out=ysb[:], in_=psy[:])
                    nc.sync.dma_start(out=y_ch[n][dsl, :], in_=ysb[:])
                nc.gpsimd.collective_compute(
                    "ReduceScatter",
                    ALU.add,
                    replica_groups=[list(range(N_CORES))],
                    ins=[y_ch[n].opt()],
                    outs=[y_rs[n].opt()],
                )
                nc.gpsimd.dma_start(out=out_d[:, nsl], in_=y_rs[n][:])

    nc.compile()
    return nc


def kernel(x, gate_w, w1, w2, w3, sw1, sw2, sw3):
    global last_exec_time_ns

    xt = np.ascontiguousarray(
        np.asarray(x, np.float32).reshape(T, DIM).T)      # [D, T] fp32
    xtb = xt.astype(BF)
    xtl = (xt - xtb.astype(np.float32)).astype(BF)
    gate_w = np.asarray(gate_w, np.float32)
    w1 = np.asarray(w1, np.float32)
    w2 = np.asarray(w2, np.float32)
    w3 = np.asarray(w3, np.float32)
    sw1 = np.asarray(sw1, np.float32)
    sw2 = np.asarray(sw2, np.float32)
    sw3 = np.asarray(sw3, np.float32)

    in_maps = []
    for c in range(N_CORES):
        mine = [EPC * c + j for j in range(EPC)]
        perm = mine + [e for e in range(E) if e not in mine]
        gwt = np.ascontiguousarray(gate_w[perm].T)        # [D, E] fp32
        gh = gwt.astype(BF)
        gl = (gwt - gh.astype(np.float32)).astype(BF)
        ghgl = np.concatenate([gh, gl], axis=1)           # [D, 2E]
        w1t = np.stack([np.ascontiguousarray(w1[e].T) for e in mine]).astype(BF)
        w3t = np.stack([np.ascontiguousarray(w3[e].T) for e in mine]).astype(BF)
        w2t = np.stack([np.ascontiguousarray(w2[e].T) for e in mine]).astype(BF)
        ssl = slice(SIC * c, SIC * (c + 1))
        sw1t = np.ascontiguousarray(sw1[ssl, :].T).astype(BF)  # [D, SIC]
        sw3t = np.ascontiguousarray(sw3[ssl, :].T).astype(BF)
        sw2t = np.ascontiguousarray(sw2[:, ssl].T).astype(BF)  # [SIC, D]
        in_maps.append({
            "xtb": xtb, "xtl": xtl, "ghgl": ghgl,
            "w1t": w1t, "w3t": w3t, "w2t": w2t,
            "sw1t": sw1t, "sw3t": sw3t, "sw2t": sw2t,
        })

    if "nc" not in _cached:
        _cached["nc"] = _build()
    nc = _cached["nc"]

    res = run_bass_kernel_spmd(nc, in_maps, core_ids=list(range(N_CORES)))
    last_exec_time_ns = res.exec_time_ns

    yt = np.concatenate([res.results[c]["out"].astype(np.float32)
                         for c in range(N_CORES)], axis=0)  # [D, T]
    return np.ascontiguousarray(yt.T).reshape(B, S, DIM).astype(np.float32)


# revision 25
# speedup vs baseline: 1.0227x; 1.0227x over previous
"""Trainium2 Bass kernel for nn_MoE_66803921322559.

Top-2-of-16 MoE (T=2048 tokens, D=1024, INTER=512) + shared expert
(SHARED_INTER=1024), expert-parallel over 8 NeuronCores:

  - core c owns experts (2c, 2c+1); gate computed on-device (double-bf16
    logits = xh@gh + xl@gh + xh@gl, exact enough that top-2 selection
    matches fp32; sigmoid/normalize for combine weights)
  - routed experts + shared-expert slice computed in bf16 on device;
    shared expert is scheduled first so the gate pipeline latency hides
  - partial outputs y [D, T] summed across cores with 4 d-chunked
    ReduceScatters overlapped with the y-phase matmuls; host reassembles.
"""

import os
import sys
import types

import numpy as np

sys.path.insert(0, "/opt/trn_rl_repo")

import ml_dtypes

BF = ml_dtypes.bfloat16

B, S, DIM = 2, 1024, 1024
E, K, INTER = 16, 2, 512
T = B * S
N_CORES = 8
EPC = E // N_CORES          # experts per core
SIC = 2 * INTER // N_CORES  # shared-inter slice per core (128)

KD = DIM // 128             # 8 contraction chunks over D
NT = T // 512               # 4 token chunks of 512
MI = INTER // 128           # 4 inter chunks per expert
NTILE = T // 128            # 16 token tiles of 128
NCH = 4                     # ReduceScatter d-chunks


def _install_ntff_hook():
    """Provide antenv.axon_hooks (missing in this container) so
    run_bass_kernel_spmd(trace=True) can capture NTFF profiles via axon."""
    try:
        import antenv
        if hasattr(antenv, "axon_hooks"):
            return
        from trn_agent_boot.trn_boot import _ntff_profile_via_ctypes
        hook = _ntff_profile_via_ctypes("/opt/axon/libaxon_pjrt.so")
        mod = types.ModuleType("antenv.axon_hooks")
        mod._hook = hook
        mod.get_axon_ntff_profile_hook = lambda: mod._hook
        mod.set_axon_ntff_profile_hook = lambda h: setattr(mod, "_hook", h)
        sys.modules["antenv.axon_hooks"] = mod
        antenv.axon_hooks = mod
    except Exception:
        pass


_install_ntff_hook()

from concourse import bacc, bass, mybir, tile  # noqa: E402
from concourse.bass_utils import run_bass_kernel_spmd  # noqa: E402
from concourse.masks import make_identity  # noqa: E402

F32 = mybir.dt.float32
BF16 = mybir.dt.bfloat16
AF = mybir.ActivationFunctionType
ALU = mybir.AluOpType

last_exec_time_ns = None
_cached = {}


def _build():
    nc = bacc.Bacc("TRN2", target_bir_lowering=False, debug=False,
                   num_devices=N_CORES)

    xtb_d = nc.dram_tensor("xtb", [DIM, T], BF16, kind="ExternalInput").ap()
    xtl_d = nc.dram_tensor("xtl", [DIM, T], BF16, kind="ExternalInput").ap()
    ghgl_d = nc.dram_tensor("ghgl", [DIM, 2 * E], BF16, kind="ExternalInput").ap()
    w1t_d = nc.dram_tensor("w1t", [EPC, DIM, INTER], BF16, kind="ExternalInput").ap()
    w3t_d = nc.dram_tensor("w3t", [EPC, DIM, INTER], BF16, kind="ExternalInput").ap()
    w2t_d = nc.dram_tensor("w2t", [EPC, INTER, DIM], BF16, kind="ExternalInput").ap()
    sw1t_d = nc.dram_tensor("sw1t", [DIM, SIC], BF16, kind="ExternalInput").ap()
    sw3t_d = nc.dram_tensor("sw3t", [DIM, SIC], BF16, kind="ExternalInput").ap()
    sw2t_d = nc.dram_tensor("sw2t", [SIC, DIM], BF16, kind="ExternalInput").ap()
    out_d = nc.dram_tensor("out", [DIM // N_CORES, T], BF16,
                           kind="ExternalOutput").ap()

    with tile.TileContext(nc) as tc:
        with (
            tc.tile_pool(name="wpool", bufs=1) as wp,
            tc.tile_pool(name="work", bufs=3) as wk,
            tc.tile_pool(name="psum", bufs=2, space="PSUM") as pp,
            tc.tile_pool(name="psc", bufs=2, space="PSUM") as pscp,
            tc.tile_pool(name="dram", bufs=1, space="DRAM") as dp,
        ):
            # ---- persistent SBUF loads, in consumption order ---------------
            # sync ring: gate inputs first; scalar ring: expert weights
            xtb, xtl, ghs = [], [], []
            for k in range(KD):
                ksl = slice(k * 128, (k + 1) * 128)
                t_ = wp.tile([128, T], BF16, tag=f"xtb{k}", name=f"xtb{k}")
                nc.sync.dma_start(out=t_[:], in_=xtb_d[ksl, :])
                xtb.append(t_)
                t_ = wp.tile([128, 2 * E], BF16, tag=f"gh{k}", name=f"gh{k}")
                nc.sync.dma_start(out=t_[:], in_=ghgl_d[ksl, :])
                ghs.append(t_)
            for k in range(KD):
                ksl = slice(k * 128, (k + 1) * 128)
                t_ = wp.tile([128, T], BF16, tag=f"xtl{k}", name=f"xtl{k}")
                nc.sync.dma_start(out=t_[:], in_=xtl_d[ksl, :])
                xtl.append(t_)
            sw1s, sw3s = [], []
            for k in range(KD):
                ksl = slice(k * 128, (k + 1) * 128)
                t_ = wp.tile([128, SIC], BF16, tag=f"sw1s{k}", name=f"sw1s{k}")
                nc.scalar.dma_start(out=t_[:], in_=sw1t_d[ksl, :])
                sw1s.append(t_)
                t_ = wp.tile([128, SIC], BF16, tag=f"sw3s{k}", name=f"sw3s{k}")
                nc.scalar.dma_start(out=t_[:], in_=sw3t_d[ksl, :])
                sw3s.append(t_)
            w1s = [[None] * KD for _ in range(EPC)]
            w3s = [[None] * KD for _ in range(EPC)]
            for e in range(EPC):
                for k in range(KD):
                    ksl = slice(k * 128, (k + 1) * 128)
                    t_ = wp.tile([128, INTER], BF16, tag=f"w1s{e}_{k}",
                                 name=f"w1s{e}_{k}")
                    nc.scalar.dma_start(out=t_[:], in_=w1t_d[e, ksl, :])
                    w1s[e][k] = t_
                    t_ = wp.tile([128, INTER], BF16, tag=f"w3s{e}_{k}",
                                 name=f"w3s{e}_{k}")
                    nc.scalar.dma_start(out=t_[:], in_=w3t_d[e, ksl, :])
                    w3s[e][k] = t_
            w2s = [[None] * MI for _ in range(EPC)]
            for e in range(EPC):
                for m in range(MI):
                    msl = slice(m * 128, (m + 1) * 128)
                    t_ = wp.tile([128, DIM], BF16, tag=f"w2s{e}_{m}",
                                 name=f"w2s{e}_{m}")
                    nc.scalar.dma_start(out=t_[:], in_=w2t_d[e, msl, :])
                    w2s[e][m] = t_
            sw2s = wp.tile([128, DIM], BF16, tag="sw2s")
            nc.scalar.dma_start(out=sw2s[:], in_=sw2t_d[:, :])

            ident = wp.tile([128, 128], F32, tag="ident")
            make_identity(nc, ident[:])

            # ---- gate: double-bf16 logits, [gh|gl] packed stationary -------
            # rows 0:16 = gh-terms, 16:32 = gl-terms; summed after transpose
            logits = wp.tile([2 * E, T], F32, tag="logits")
            for n in range(NT):
                nsl = slice(n * 512, (n + 1) * 512)
                psc = pscp.tile([2 * E, 512], F32, tag="pg", name="psc")
                for k in range(KD):
                    nc.tensor.matmul(psc[:], lhsT=ghs[k][:], rhs=xtb[k][:, nsl],
                                     start=(k == 0), stop=False)
                for k in range(KD):
                    nc.tensor.matmul(psc[:], lhsT=ghs[k][:], rhs=xtl[k][:, nsl],
                                     start=False, stop=(k == KD - 1))
                nc.vector.tensor_copy(out=logits[:, nsl], in_=psc[:])

            # ---- shared expert slice (no gate dependency): emitted before
            # the gate's transpose/DVE chain so PE stays busy ----------------
            hss = wp.tile([128, T], BF16, tag="hss")
            for n in range(NT):
                nsl = slice(n * 512, (n + 1) * 512)
                ps1 = pp.tile([128, 512], F32, tag="ps1", name="ps1")
                ps3 = pp.tile([128, 512], F32, tag="ps3", name="ps3")
                for k in range(KD):
                    nc.tensor.matmul(ps1[:], lhsT=sw1s[k][:], rhs=xtb[k][:, nsl],
                                     start=(k == 0), stop=(k == KD - 1))
                for k in range(KD):
                    nc.tensor.matmul(ps3[:], lhsT=sw3s[k][:], rhs=xtb[k][:, nsl],
                                     start=(k == 0), stop=(k == KD - 1))
                t1 = wk.tile([128, 512], BF16, tag="t1")
                nc.scalar.activation(out=t1[:], in_=ps1[:], func=AF.Silu)
                nc.vector.tensor_tensor(out=hss[:, nsl], in0=t1[:], in1=ps3[:],
                                        op=ALU.mult)

            # batched gate: transpose logits, top-2, combine weights
            lg_all = wp.tile([128, NTILE * E], F32, tag="lg_all")
            mx_all = wp.tile([128, NTILE * 8], F32, tag="mx_all")
            for i in range(NTILE):
                isl = slice(i * 128, (i + 1) * 128)
                ptr = pscp.tile([128, 2 * E], F32, tag="pg", name="ptr")
                nc.tensor.transpose(out=ptr[:], in_=logits[:, isl],
                                    identity=ident[:2 * E, :2 * E])
                lg32 = wk.tile([128, 2 * E], F32, tag="lg32")
                nc.vector.tensor_copy(out=lg32[:], in_=ptr[:])
                nc.vector.tensor_tensor(out=lg_all[:, i * E:(i + 1) * E],
                                        in0=lg32[:, 0:E], in1=lg32[:, E:2 * E],
                                        op=ALU.add)
            for i in range(NTILE):
                nc.vector.max(out=mx_all[:, i * 8:(i + 1) * 8],
                              in_=lg_all[:, i * E:(i + 1) * E])
            # denominator = sigmoid(l1) + sigmoid(l2); rec = 1/den
            mx3 = mx_all[:].rearrange("p (i c) -> p i c", c=8)
            lg3 = lg_all[:].rearrange("p (i c) -> p i c", c=E)
            s12 = wk.tile([128, 2 * NTILE], F32, tag="s12")
            nc.scalar.activation(out=s12[:], in_=mx3[:, :, 0:2], func=AF.Sigmoid)
            s12v = s12[:].rearrange("p (i c) -> p i c", c=2)
            den = wk.tile([128, NTILE], F32, tag="den")
            nc.vector.tensor_tensor(out=den[:], in0=s12v[:, :, 0],
                                    in1=s12v[:, :, 1], op=ALU.add)
            rec = wk.tile([128, NTILE], F32, tag="rec")
            nc.vector.reciprocal(out=rec[:], in_=den[:])

            wexp = []
            for e in range(EPC):
                sel = wk.tile([128, NTILE], F32, tag=f"sel{e}", name=f"sel{e}")
                nc.vector.tensor_tensor(out=sel[:], in0=lg3[:, :, e],
                                        in1=mx3[:, :, 1], op=ALU.is_ge)
                sg = wk.tile([128, NTILE], F32, tag=f"sg{e}", name=f"sg{e}")
                nc.scalar.activation(out=sg[:], in_=lg3[:, :, e], func=AF.Sigmoid)
                we = wk.tile([128, NTILE], F32, tag=f"wexp{e}", name=f"wexp{e}")
                nc.vector.tensor_tensor(out=we[:], in0=sg[:], in1=sel[:],
                                        op=ALU.mult)
                nc.vector.tensor_tensor(out=we[:], in0=we[:], in1=rec[:],
                                        op=ALU.mult)
                wexp.append(we)

            # transpose per-token weights into rows, then partition-broadcast
            wbc = []
            for e in range(EPC):
                wrow = wp.tile([1, T], BF16, tag=f"wrow{e}", name=f"wrow{e}")
                for i in range(NTILE):
                    pwt = pscp.tile([1, 128], F32, tag="pg", name="pwt")
                    nc.tensor.transpose(out=pwt[:], in_=wexp[e][:, i:i + 1],
                                        identity=ident[:])
                    nc.vector.tensor_copy(out=wrow[:, i * 128:(i + 1) * 128],
                                          in_=pwt[:])
                t_ = wp.tile([128, T], BF16, tag=f"wbc{e}", name=f"wbc{e}")
                nc.gpsimd.partition_broadcast(t_[:], wrow[:, :])
                wbc.append(t_)

            # ---- fused pipeline, token-chunk outer: h(n) -> y(n) -> RS(n) --
            hsb = [[wp.tile([128, T], BF16, tag=f"hsb{e}_{m}", name=f"hsb{e}_{m}")
                    for m in range(MI)] for e in range(EPC)]
            y_ch = [dp.tile([DIM, 512], BF16, name=f"y_ch{n}") for n in range(NT)]
            y_rs = [dp.tile([DIM // N_CORES, 512], BF16, name=f"y_rs{n}")
                    for n in range(NT)]
            for n in range(NT):
                nsl = slice(n * 512, (n + 1) * 512)
                # routed experts
                for e in range(EPC):
                    for m in range(MI):
                        msl = slice(m * 128, (m + 1) * 128)
                        ps1 = pp.tile([128, 512], F32, tag="ps1", name="ps1")
                        ps3 = pp.tile([128, 512], F32, tag="ps3", name="ps3")
                        for k in range(KD):
                            nc.tensor.matmul(ps1[:], lhsT=w1s[e][k][:, msl],
                                             rhs=xtb[k][:, nsl],
                                             start=(k == 0), stop=(k == KD - 1))
                        for k in range(KD):
                            nc.tensor.matmul(ps3[:], lhsT=w3s[e][k][:, msl],
                                             rhs=xtb[k][:, nsl],
                                             start=(k == 0), stop=(k == KD - 1))
                        t1 = wk.tile([128, 512], BF16, tag="t1")
                        nc.scalar.activation(out=t1[:], in_=ps1[:], func=AF.Silu)
                        nc.vector.tensor_tensor(out=hsb[e][m][:, nsl], in0=t1[:],
                                                in1=ps3[:], op=ALU.mult)
                # apply gate weights in place, decoupled from the PSUM pipeline
                for e in range(EPC):
                    for m in range(MI):
                        nc.vector.tensor_tensor(out=hsb[e][m][:, nsl],
                                                in0=hsb[e][m][:, nsl],
                                                in1=wbc[e][:, nsl], op=ALU.mult)
                # y for this token chunk, then its ReduceScatter
                for d in range(KD):
                    dsl = slice(d * 128, (d + 1) * 128)
                    psy = pp.tile([128, 512], F32, tag="psy", name="psy")
                    nc.tensor.matmul(psy[:], lhsT=sw2s[:, dsl],
                                     rhs=hss[:, nsl], start=True, stop=False)
                    for e in range(EPC):
                        for m in range(MI):
                            nc.tensor.matmul(
                                psy[:], lhsT=w2s[e][m][:, dsl],
                                rhs=hsb[e][m][:, nsl],
                                start=False,
                                stop=(e == EPC - 1 and m == MI - 1))
                    ysb = wk.tile([128, 512], BF16, tag="ysb")
                    nc.vector.tensor_copy(out=ysb[:], in_=psy[:])
                    nc.sync.dma_start(out=y_ch[n][dsl, :], in_=ysb[:])
                nc.gpsimd.collective_compute(
                    "ReduceScatter",
                    ALU.add,
                    replica_groups=[list(range(N_CORES))],
                    ins=[y_ch[n].opt()],
                    outs=[y_rs[n].opt()],
                )
                nc.gpsimd.dma_start(out=out_d[:, nsl], in_=y_rs[n][:])

    nc.compile()
    return nc


def kernel(x, gate_w, w1, w2, w3, sw1, sw2, sw3):
    global last_exec_time_ns

    xt = np.ascontiguousarray(
        np.asarray(x, np.float32).reshape(T, DIM).T)      # [D, T] fp32
    xtb = xt.astype(BF)
    xtl = (xt - xtb.astype(np.float32)).astype(BF)
    gate_w = np.asarray(gate_w, np.float32)
    w1 = np.asarray(w1, np.float32)
    w2 = np.asarray(w2, np.float32)
    w3 = np.asarray(w3, np.float32)
    sw1 = np.asarray(sw1, np.float32)
    sw2 = np.asarray(sw2, np.float32)
    sw3 = np.asarray(sw3, np.float32)

    in_maps = []
    for c in range(N_CORES):
        mine = [EPC * c + j for j in range(EPC)]
        perm = mine + [e for e in range(E) if e not in mine]
        gwt = np.ascontiguousarray(gate_w[perm].T)        # [D, E] fp32
        gh = gwt.astype(BF)
        gl = (gwt - gh.astype(np.float32)).astype(BF)
        ghgl = np.concatenate([gh, gl], axis=1)           # [D, 2E]
        w1t = np.stack([np.ascontiguousarray(w1[e].T) for e in mine]).astype(BF)
        w3t = np.stack([np.ascontiguousarray(w3[e].T) for e in mine]).astype(BF)
        w2t = np.stack([np.ascontiguousarray(w2[e].T) for e in mine]).astype(BF)
        ssl = slice(SIC * c, SIC * (c + 1))
        sw1t = np.ascontiguousarray(sw1[ssl, :].T).astype(BF)  # [D, SIC]
        sw3t = np.ascontiguousarray(sw3[ssl, :].T).astype(BF)
        sw2t = np.ascontiguousarray(sw2[:, ssl].T).astype(BF)  # [SIC, D]
        in_maps.append({
            "xtb": xtb, "xtl": xtl, "ghgl": ghgl,
            "w1t": w1t, "w3t": w3t, "w2t": w2t,
            "sw1t": sw1t, "sw3t": sw3t, "sw2t": sw2t,
        })

    if "nc" not in _cached:
        _cached["nc"] = _build()
    nc = _cached["nc"]

    res = run_bass_kernel_spmd(nc, in_maps, core_ids=list(range(N_CORES)))
    last_exec_time_ns = res.exec_time_ns

    yt = np.concatenate([res.results[c]["out"].astype(np.float32)
                         for c in range(N_CORES)], axis=0)  # [D, T]
    return np.ascontiguousarray(yt.T).reshape(B, S, DIM).astype(np.float32)
